# revision 11
# baseline (speedup 1.0000x reference)
"""Trainium2 Bass kernel for nn_AttentionBlock (B=32, C=1024, H=W=32, nh=1).

Reference computation (per batch b, with S = H*W = 1024):
    qkv = w_qkv @ x_b            # [3C, S], 1x1 conv == channel matmul
    q, k, v = split(qkv)
    logits[t,s] = (q[:,t] . k[:,s]) / sqrt(C)
    attn = softmax_s(logits)
    h[t,s] = attn[t,s] * sum_c v[c,s]
    out = w_proj @ h + b_proj + x_b

Algebraic simplifications (all weight-only, precomputed on host):
  * logits = x^T (M x) with M = Wq^T Wk  -> q/k never materialized.
  * sum_c v[c,s] = (sum_c Wv[c,:]) . x[:,s] -> single row 'wvs'.
  * h = attn .* (1 vs^T).

Matmuls run in float32r (single-pass fp32 on the PE = 4x fp32 rate).
float32r keeps 11 mantissa bits; inputs are pre-rounded on the host, and
on-chip producers write float32r tiles so the engines round. The residual
path uses exact fp32 x chunks streamed separately.

Sharding: data-parallel over batch, 4 batches per core on 8 cores.
"""

import os
import sys

import numpy as np

for _p in ("/opt/trn_rl_repo", "/opt/pypackages"):
    if _p not in sys.path:
        sys.path.insert(0, _p)

import concourse.bass as bass
import concourse.tile as tile
from concourse import bacc, mybir
from concourse.bass_utils import run_bass_kernel_spmd

B, C, HH, WW = 32, 1024, 32, 32
S = HH * WW          # 1024 spatial positions
P = 128              # partitions
KC = C // P          # 8 chunks along channel dim
TC = S // P          # 8 chunks along spatial (t) dim
NN = 512             # matmul moving free dim
NCH = S // NN        # 2 free-dim halves
N_CORES = 8
BPC = B // N_CORES   # batches per core
SCALE = 1.0 / np.sqrt(float(C))  # folded into the exp

f32 = mybir.dt.float32
f32r = mybir.dt.float32r


def build_nc(bpc: int = BPC):
    nc = bacc.Bacc(
        "TRN2",
        target_bir_lowering=False,
        debug=False,
        enable_asserts=False,
    )

    x_d = nc.dram_tensor("x", [bpc, C, S], f32r, kind="ExternalInput")
    xf_d = nc.dram_tensor("xf", [bpc, C, S], f32, kind="ExternalInput")
    mt_d = nc.dram_tensor("mt", [C, C], f32r, kind="ExternalInput")    # (Wq^T Wk)^T
    wpt_d = nc.dram_tensor("wpt", [C, C], f32r, kind="ExternalInput")  # w_proj^T
    wvs_d = nc.dram_tensor("wvs", [C], f32r, kind="ExternalInput")     # sum Wv rows
    bp_d = nc.dram_tensor("bp", [C], f32, kind="ExternalInput")        # b_proj
    out_d = nc.dram_tensor("out", [bpc, C, S], f32, kind="ExternalOutput")

    with tile.TileContext(nc) as tc:
        with (
            tc.tile_pool(name="weights", bufs=1) as wpool,
            tc.tile_pool(name="xc", bufs=10) as xpool,
            tc.tile_pool(name="xf", bufs=4) as xfpool,
            tc.tile_pool(name="y", bufs=1) as ypool,
            tc.tile_pool(name="h", bufs=9) as hpool,
            tc.tile_pool(name="vsb", bufs=2) as vpool,
            tc.tile_pool(name="osb", bufs=4) as opool,
            tc.tile_pool(name="small", bufs=16) as spool,
            tc.tile_pool(name="psA", bufs=2, space="PSUM") as psA,
            tc.tile_pool(name="psB", bufs=2, space="PSUM") as psB,
            tc.tile_pool(name="psC", bufs=2, space="PSUM") as psC,
        ):
            # ---- resident weights ----
            mt_sb = wpool.tile([P, KC, C], f32r, tag="mt")
            nc.sync.dma_start(mt_sb[:], mt_d.rearrange("(ko ki) m -> ki ko m", ki=P))
            wpt_sb = wpool.tile([P, TC, C], f32r, tag="wpt")
            nc.sync.dma_start(wpt_sb[:], wpt_d.rearrange("(ko ki) m -> ki ko m", ki=P))
            wvs_sb = wpool.tile([P, KC], f32r, tag="wvs")
            nc.sync.dma_start(wvs_sb[:], wvs_d.rearrange("(ko ki) -> ki ko", ki=P))
            bp_sb = wpool.tile([P, KC], f32, tag="bp")
            nc.sync.dma_start(bp_sb[:], bp_d.rearrange("(o p) -> p o", p=P))
            # wvs replicated across 128 weight columns: the vs-matmul then
            # emits vs[s] identically on all 128 PSUM partitions.
            wvs_rep = wpool.tile([P, KC, P], f32r, tag="wvs_rep")
            nc.vector.tensor_copy(
                out=wvs_rep[:], in_=wvs_sb[:, :, None].to_broadcast([P, KC, P])
            )

            for b in range(bpc):
                # ---- load x chunks (f32r, matmul operand) ----
                xc = []
                for k in range(KC):
                    t = xpool.tile([P, S], f32r, tag="xc")
                    nc.sync.dma_start(t[:], x_d[b, k * P : (k + 1) * P, :])
                    xc.append(t)

                # ---- stage A: y = M x  (y[c',s]) ----
                y_sb = ypool.tile([P, KC, S], f32r, tag="y")
                for mc in range(KC):
                    for n in range(NCH):
                        ps = psA.tile([P, NN], f32, tag="psA")
                        for k in range(KC):
                            nc.tensor.matmul(
                                ps[:],
                                mt_sb[:, k, mc * P : (mc + 1) * P],
                                xc[k][:, n * NN : (n + 1) * NN],
                                start=(k == 0),
                                stop=(k == KC - 1),
                            )
                        nc.any.tensor_copy(
                            out=y_sb[:, mc, n * NN : (n + 1) * NN], in_=ps[:]
                        )

                # ---- stage A2: vs[s] = wvs . x[:,s] on all 128 partitions ----
                vsb = vpool.tile([P, S], f32r, tag="vsb")
                for n in range(NCH):
                    psv = psA.tile([P, NN], f32, tag="psA")
                    for k in range(KC):
                        nc.tensor.matmul(
                            psv[:],
                            wvs_rep[:, k, :],
                            xc[k][:, n * NN : (n + 1) * NN],
                            start=(k == 0),
                            stop=(k == KC - 1),
                        )
                    nc.any.tensor_copy(
                        out=vsb[:, n * NN : (n + 1) * NN], in_=psv[:]
                    )

                # ---- stage B: logits tiles, fused softmax * vs ----
                hts = []
                for tt in range(TC):
                    psl = psB.tile([P, S], f32, tag="psB")
                    for n in range(NCH):
                        for k in range(KC):
                            nc.tensor.matmul(
                                psl[:, n * NN : (n + 1) * NN],
                                xc[k][:, tt * P : (tt + 1) * P],
                                y_sb[:, k, n * NN : (n + 1) * NN],
                                start=(k == 0),
                                stop=(k == KC - 1),
                            )
                    e = hpool.tile([P, S], f32r, tag="h")
                    rs = spool.tile([P, 1], f32, tag="rs")
                    # e = exp(logits / sqrt(C)); rs = row sums (free on ACT)
                    nc.scalar.activation(
                        e[:], psl[:], mybir.ActivationFunctionType.Exp,
                        scale=float(SCALE), accum_out=rs[:],
                    )
                    rcp = spool.tile([P, 1], f32, tag="rcp")
                    nc.vector.reciprocal(rcp[:], rs[:])
                    # normalize rows (per-partition scalar) on ACT
                    nc.scalar.activation(
                        e[:], e[:], mybir.ActivationFunctionType.Copy,
                        scale=rcp[:],
                    )
                    # h = attn * vs  (vs broadcast over partitions via vsb)
                    nc.vector.tensor_tensor(
                        e[:], e[:], vsb[:], mybir.AluOpType.mult
                    )
                    hts.append(e)

                # ---- stage C: out = w_proj @ h + x + b ----
                for oc in range(KC):
                    for n in range(NCH):
                        pso = psC.tile([P, NN], f32, tag="psC")
                        for tt in range(TC):
                            nc.tensor.matmul(
                                pso[:],
                                wpt_sb[:, tt, oc * P : (oc + 1) * P],
                                hts[tt][:, n * NN : (n + 1) * NN],
                                start=(tt == 0),
                                stop=(tt == TC - 1),
                            )
                        xf_t = xfpool.tile([P, NN], f32, tag="xf")
                        nc.sync.dma_start(
                            xf_t[:],
                            xf_d[b, oc * P : (oc + 1) * P, n * NN : (n + 1) * NN],
                        )
                        osb = opool.tile([P, NN], f32, tag="osb")
                        nc.vector.tensor_tensor(
                            osb[:], pso[:], xf_t[:], mybir.AluOpType.add
                        )
                        nc.vector.tensor_scalar(
                            osb[:], osb[:], bp_sb[:, oc : oc + 1], None,
                            mybir.AluOpType.add,
                        )
                        nc.sync.dma_start(
                            out_d[b, oc * P : (oc + 1) * P, n * NN : (n + 1) * NN],
                            osb[:],
                        )
    nc.compile()
    return nc


def _round_f32r(a):
    """Round fp32 to float32r (11-bit mantissa, round-to-nearest-even)."""
    u = np.ascontiguousarray(a.astype(np.float32)).view(np.uint32)
    lsb = (u >> np.uint32(12)) & np.uint32(1)
    r = (u + np.uint32(0x7FF) + lsb) & np.uint32(0xFFFFF000)
    return r.view(np.float32)


def _host_prep(w_qkv, w_proj, b_proj):
    wq = w_qkv[0:C].astype(np.float64)
    wk = w_qkv[C : 2 * C].astype(np.float64)
    wv = w_qkv[2 * C : 3 * C]
    # lhsT for y-matmul: MT[c, c'] = M[c', c],  M = Wq^T Wk  =>  MT = Wk^T Wq
    mt = np.ascontiguousarray(wk.T @ wq).astype(np.float32)
    wvs = wv.sum(axis=0, dtype=np.float64).astype(np.float32)
    wpt = np.ascontiguousarray(w_proj.T).astype(np.float32)
    return _round_f32r(mt), _round_f32r(wpt), _round_f32r(wvs), b_proj.astype(np.float32)


_NC_CACHE = {}


def _get_nc(bpc=BPC):
    if bpc not in _NC_CACHE:
        _NC_CACHE[bpc] = build_nc(bpc)
    return _NC_CACHE[bpc]


def kernel(x, w_qkv, w_proj, b_proj, _trace=False):
    x = np.asarray(x, dtype=np.float32)
    mt, wpt, wvs, bp = _host_prep(
        np.asarray(w_qkv, np.float32),
        np.asarray(w_proj, np.float32),
        np.asarray(b_proj, np.float32),
    )
    xr_full = x.reshape(B, C, S)
    x_rounded = _round_f32r(xr_full)
    in_maps = []
    for c in range(N_CORES):
        sl = slice(c * BPC, (c + 1) * BPC)
        in_maps.append(
            {
                "x": np.ascontiguousarray(x_rounded[sl]),
                "xf": np.ascontiguousarray(xr_full[sl]),
                "mt": mt,
                "wpt": wpt,
                "wvs": wvs,
                "bp": bp,
            }
        )
    nc = _get_nc()
    res = run_bass_kernel_spmd(
        nc, in_maps, core_ids=list(range(N_CORES)), trace=_trace
    )
    out = np.concatenate([r["out"] for r in res.results], axis=0)
    out = out.reshape(B, C, HH, WW)
    if _trace:
        kernel.last_results = res
    return out


# revision 12
# speedup vs baseline: 1.0106x; 1.0106x over previous
"""Trainium2 Bass kernel for nn_AttentionBlock (B=32, C=1024, H=W=32, nh=1).

Reference computation (per batch b, with S = H*W = 1024):
    qkv = w_qkv @ x_b            # [3C, S], 1x1 conv == channel matmul
    q, k, v = split(qkv)
    logits[t,s] = (q[:,t] . k[:,s]) / sqrt(C)
    attn = softmax_s(logits)
    h[t,s] = attn[t,s] * sum_c v[c,s]
    out = w_proj @ h + b_proj + x_b

Algebraic simplifications (all weight-only, precomputed on host):
  * logits = x^T (M x) with M = Wq^T Wk  -> q/k never materialized.
  * sum_c v[c,s] = (sum_c Wv[c,:]) . x[:,s] -> single row 'wvs'.
  * h = attn .* (1 vs^T).

Matmul dtype is switchable (float32r or bfloat16); the residual path
always uses exact fp32 x chunks streamed separately, so output error is
dominated by the small projection term only.

Sharding: data-parallel over batch, 4 batches per core on 8 cores.
"""

import os
import sys

import numpy as np

for _p in ("/opt/trn_rl_repo", "/opt/pypackages"):
    if _p not in sys.path:
        sys.path.insert(0, _p)

import ml_dtypes

import concourse.bass as bass
import concourse.tile as tile
from concourse import bacc, mybir
from concourse.bass_utils import run_bass_kernel_spmd

B, C, HH, WW = 32, 1024, 32, 32
S = HH * WW          # 1024 spatial positions
P = 128              # partitions
KC = C // P          # 8 chunks along channel dim
TC = S // P          # 8 chunks along spatial (t) dim
NN = 512             # matmul moving free dim
NCH = S // NN        # 2 free-dim halves
N_CORES = 8
BPC = B // N_CORES   # batches per core
SCALE = 1.0 / np.sqrt(float(C))  # folded into the exp

f32 = mybir.dt.float32
f32r = mybir.dt.float32r
bf16 = mybir.dt.bfloat16

MM_DTYPE = os.environ.get("KERNEL_MM_DTYPE", "f32r")


def build_nc(bpc: int = BPC, mmdt_name: str | None = None):
    mmdt = {"f32r": f32r, "bf16": bf16}[mmdt_name or MM_DTYPE]
    nc = bacc.Bacc(
        "TRN2",
        target_bir_lowering=False,
        debug=False,
        enable_asserts=False,
    )

    x_d = nc.dram_tensor("x", [bpc, C, S], mmdt, kind="ExternalInput")
    xf_d = nc.dram_tensor("xf", [bpc, C, S], f32, kind="ExternalInput")
    # weight stripes pre-arranged on host: [chunk, c, 128]
    mt_d = nc.dram_tensor("mt", [KC, C, P], mmdt, kind="ExternalInput")
    wpt_d = nc.dram_tensor("wpt", [KC, C, P], mmdt, kind="ExternalInput")
    wvs_d = nc.dram_tensor("wvs", [C], mmdt, kind="ExternalInput")
    bp_d = nc.dram_tensor("bp", [C], f32, kind="ExternalInput")
    out_d = nc.dram_tensor("out", [bpc, C, S], f32, kind="ExternalOutput")

    with tile.TileContext(nc) as tc:
        with (
            tc.tile_pool(name="weights", bufs=1) as wpool,
            tc.tile_pool(name="xc", bufs=10) as xpool,
            tc.tile_pool(name="xf", bufs=4) as xfpool,
            tc.tile_pool(name="y", bufs=1) as ypool,
            tc.tile_pool(name="h", bufs=9) as hpool,
            tc.tile_pool(name="vsb", bufs=2) as vpool,
            tc.tile_pool(name="osb", bufs=4) as opool,
            tc.tile_pool(name="small", bufs=16) as spool,
            tc.tile_pool(name="psA", bufs=2, space="PSUM") as psA,
            tc.tile_pool(name="psB", bufs=2, space="PSUM") as psB,
            tc.tile_pool(name="psC", bufs=2, space="PSUM") as psC,
        ):
            # ---- small resident weights first (cheap DMAs) ----
            wvs_sb = wpool.tile([P, KC], mmdt, tag="wvs")
            nc.sync.dma_start(wvs_sb[:], wvs_d.rearrange("(ko ki) -> ki ko", ki=P))
            bp_sb = wpool.tile([P, KC], f32, tag="bp")
            nc.sync.dma_start(bp_sb[:], bp_d.rearrange("(o p) -> p o", p=P))
            # wvs replicated across 128 weight columns: the vs-matmul then
            # emits vs[s] identically on all 128 PSUM partitions.
            wvs_rep = wpool.tile([P, KC, P], mmdt, tag="wvs_rep")
            nc.vector.tensor_copy(
                out=wvs_rep[:], in_=wvs_sb[:, :, None].to_broadcast([P, KC, P])
            )
            # mt stripes: per-mc contiguous loads so compute starts after
            # the first stripe, not the whole 4 MiB matrix.
            mt_sb = wpool.tile([P, KC, C], mmdt, tag="mt")
            for mc in range(KC):
                nc.sync.dma_start(
                    mt_sb[:, :, mc * P : (mc + 1) * P],
                    mt_d[mc].rearrange("(ko ki) m -> ki ko m", ki=P),
                )
            wpt_sb = wpool.tile([P, TC, C], mmdt, tag="wpt")

            for b in range(bpc):
                # ---- load x chunks (matmul operand dtype) ----
                xc = []
                for k in range(KC):
                    t = xpool.tile([P, S], mmdt, tag="xc")
                    nc.sync.dma_start(t[:], x_d[b, k * P : (k + 1) * P, :])
                    xc.append(t)

                # ---- stage A: y = M x  (y[c',s]) ----
                y_sb = ypool.tile([P, KC, S], mmdt, tag="y")
                for mc in range(KC):
                    for n in range(NCH):
                        ps = psA.tile([P, NN], f32, tag="psA")
                        for k in range(KC):
                            nc.tensor.matmul(
                                ps[:],
                                mt_sb[:, k, mc * P : (mc + 1) * P],
                                xc[k][:, n * NN : (n + 1) * NN],
                                start=(k == 0),
                                stop=(k == KC - 1),
                            )
                        nc.any.tensor_copy(
                            out=y_sb[:, mc, n * NN : (n + 1) * NN], in_=ps[:]
                        )

                # ---- stage A2: vs[s] = wvs . x[:,s] on all 128 partitions ----
                vsb = vpool.tile([P, S], mmdt, tag="vsb")
                for n in range(NCH):
                    psv = psA.tile([P, NN], f32, tag="psA")
                    for k in range(KC):
                        nc.tensor.matmul(
                            psv[:],
                            wvs_rep[:, k, :],
                            xc[k][:, n * NN : (n + 1) * NN],
                            start=(k == 0),
                            stop=(k == KC - 1),
                        )
                    nc.any.tensor_copy(
                        out=vsb[:, n * NN : (n + 1) * NN], in_=psv[:]
                    )

                if b == 0:
                    # proj weights not needed until stage C; loading them here
                    # keeps the critical-path DMAs (mt, x) uncontended.
                    for oc in range(KC):
                        nc.sync.dma_start(
                            wpt_sb[:, :, oc * P : (oc + 1) * P],
                            wpt_d[oc].rearrange("(ko ki) m -> ki ko m", ki=P),
                        )

                # ---- stage B: logits tiles, fused softmax * vs ----
                hts = []
                for tt in range(TC):
                    psl = psB.tile([P, S], f32, tag="psB")
                    for n in range(NCH):
                        for k in range(KC):
                            nc.tensor.matmul(
                                psl[:, n * NN : (n + 1) * NN],
                                xc[k][:, tt * P : (tt + 1) * P],
                                y_sb[:, k, n * NN : (n + 1) * NN],
                                start=(k == 0),
                                stop=(k == KC - 1),
                            )
                    e = hpool.tile([P, S], mmdt, tag="h")
                    rs = spool.tile([P, 1], f32, tag="rs")
                    # e = exp(logits / sqrt(C)); rs = row sums (free on ACT)
                    nc.scalar.activation(
                        e[:], psl[:], mybir.ActivationFunctionType.Exp,
                        scale=float(SCALE), accum_out=rs[:],
                    )
                    rcp = spool.tile([P, 1], f32, tag="rcp")
                    nc.vector.reciprocal(rcp[:], rs[:])
                    # normalize rows (per-partition scalar) on ACT
                    nc.scalar.activation(
                        e[:], e[:], mybir.ActivationFunctionType.Copy,
                        scale=rcp[:],
                    )
                    # h = attn * vs  (vs broadcast over partitions via vsb)
                    nc.vector.tensor_tensor(
                        e[:], e[:], vsb[:], mybir.AluOpType.mult
                    )
                    hts.append(e)

                # ---- stage C: out = w_proj @ h + x + b ----
                for oc in range(KC):
                    for n in range(NCH):
                        pso = psC.tile([P, NN], f32, tag="psC")
                        for tt in range(TC):
                            nc.tensor.matmul(
                                pso[:],
                                wpt_sb[:, tt, oc * P : (oc + 1) * P],
                                hts[tt][:, n * NN : (n + 1) * NN],
                                start=(tt == 0),
                                stop=(tt == TC - 1),
                            )
                        xf_t = xfpool.tile([P, NN], f32, tag="xf")
                        nc.sync.dma_start(
                            xf_t[:],
                            xf_d[b, oc * P : (oc + 1) * P, n * NN : (n + 1) * NN],
                        )
                        osb = opool.tile([P, NN], f32, tag="osb")
                        nc.vector.tensor_tensor(
                            osb[:], pso[:], xf_t[:], mybir.AluOpType.add
                        )
                        nc.vector.tensor_scalar(
                            osb[:], osb[:], bp_sb[:, oc : oc + 1], None,
                            mybir.AluOpType.add,
                        )
                        nc.sync.dma_start(
                            out_d[b, oc * P : (oc + 1) * P, n * NN : (n + 1) * NN],
                            osb[:],
                        )
    nc.compile()
    return nc


def _round_f32r(a):
    """Round fp32 to float32r (11-bit mantissa, round-to-nearest-even)."""
    u = np.ascontiguousarray(a.astype(np.float32)).view(np.uint32)
    lsb = (u >> np.uint32(12)) & np.uint32(1)
    r = (u + np.uint32(0x7FF) + lsb) & np.uint32(0xFFFFF000)
    return r.view(np.float32)


def _to_mmdt(a, mmdt_name):
    if mmdt_name == "bf16":
        return np.ascontiguousarray(a).astype(ml_dtypes.bfloat16)
    return _round_f32r(a)


def _host_prep(w_qkv, w_proj, b_proj, mmdt_name):
    wq = w_qkv[0:C].astype(np.float64)
    wk = w_qkv[C : 2 * C].astype(np.float64)
    wv = w_qkv[2 * C : 3 * C]
    # lhsT for y-matmul: MT[c, c'] = M[c', c],  M = Wq^T Wk  =>  MT = Wk^T Wq
    mt = np.ascontiguousarray(wk.T @ wq).astype(np.float32)
    wvs = wv.sum(axis=0, dtype=np.float64).astype(np.float32)
    wpt = np.ascontiguousarray(w_proj.T).astype(np.float32)
    # stripe layout [chunk, c, 128]
    mt_s = np.ascontiguousarray(mt.reshape(C, KC, P).transpose(1, 0, 2))
    wpt_s = np.ascontiguousarray(wpt.reshape(C, KC, P).transpose(1, 0, 2))
    return (
        _to_mmdt(mt_s, mmdt_name),
        _to_mmdt(wpt_s, mmdt_name),
        _to_mmdt(wvs, mmdt_name),
        b_proj.astype(np.float32),
    )


_NC_CACHE = {}


def _get_nc(bpc=BPC, mmdt_name=None):
    key = (bpc, mmdt_name or MM_DTYPE)
    if key not in _NC_CACHE:
        _NC_CACHE[key] = build_nc(bpc, mmdt_name)
    return _NC_CACHE[key]


def kernel(x, w_qkv, w_proj, b_proj, _trace=False):
    mmdt_name = MM_DTYPE
    x = np.asarray(x, dtype=np.float32)
    mt, wpt, wvs, bp = _host_prep(
        np.asarray(w_qkv, np.float32),
        np.asarray(w_proj, np.float32),
        np.asarray(b_proj, np.float32),
        mmdt_name,
    )
    xr_full = x.reshape(B, C, S)
    x_mm = _to_mmdt(xr_full, mmdt_name)
    in_maps = []
    for c in range(N_CORES):
        sl = slice(c * BPC, (c + 1) * BPC)
        in_maps.append(
            {
                "x": np.ascontiguousarray(x_mm[sl]),
                "xf": np.ascontiguousarray(xr_full[sl]),
                "mt": mt,
                "wpt": wpt,
                "wvs": wvs,
                "bp": bp,
            }
        )
    nc = _get_nc(BPC, mmdt_name)
    res = run_bass_kernel_spmd(
        nc, in_maps, core_ids=list(range(N_CORES)), trace=_trace
    )
    out = np.concatenate([r["out"] for r in res.results], axis=0)
    out = out.reshape(B, C, HH, WW)
    if _trace:
        kernel.last_results = res
    return out


# revision 16
# speedup vs baseline: 1.3717x; 1.3574x over previous
"""Trainium2 Bass kernel for nn_AttentionBlock (B=32, C=1024, H=W=32, nh=1).

Reference computation (per batch b, with S = H*W = 1024):
    qkv = w_qkv @ x_b            # [3C, S], 1x1 conv == channel matmul
    q, k, v = split(qkv)
    logits[t,s] = (q[:,t] . k[:,s]) / sqrt(C)
    attn = softmax_s(logits)
    h[t,s] = attn[t,s] * sum_c v[c,s]
    out = w_proj @ h + b_proj + x_b

Algebraic simplifications (all weight-only, precomputed on host):
  * logits = x^T (M x) with M = Wq^T Wk  -> q/k never materialized.
  * sum_c v[c,s] = (sum_c Wv[c,:]) . x[:,s] = vs, computed on the
    vector/scalar engines + one ones-matmul (partition reduction).
  * h = attn .* (1 vs^T).

Precision: stages A/B (everything feeding the softmax) run in float32r
(single-pass fp32 matmul, 11 mantissa bits). Stage C (projection of the
small attention term) runs in bf16 — its output is ~13% of |out|, so the
extra rounding is negligible. The residual adds exact fp32 x.

Sharding: data-parallel over batch, 4 batches per core on 8 cores.
"""

import os
import sys

import numpy as np

for _p in ("/opt/trn_rl_repo", "/opt/pypackages"):
    if _p not in sys.path:
        sys.path.insert(0, _p)

import ml_dtypes

import concourse.bass as bass
import concourse.tile as tile
from concourse import bacc, mybir
from concourse.bass_utils import run_bass_kernel_spmd

B, C, HH, WW = 32, 1024, 32, 32
S = HH * WW          # 1024 spatial positions
P = 128              # partitions
KC = C // P          # 8 chunks along channel dim
TC = S // P          # 8 chunks along spatial (t) dim
NN = 512             # matmul moving free dim
NCH = S // NN        # 2 free-dim halves
N_CORES = 8
BPC = B // N_CORES   # batches per core
SCALE = 1.0 / np.sqrt(float(C))  # folded into the exp

f32 = mybir.dt.float32
f32r = mybir.dt.float32r
bf16 = mybir.dt.bfloat16

# dtype config: "f32r" (A/B f32r + C bf16), "all_f32r", "bf16" (everything)
MM_CFG = os.environ.get("KERNEL_MM_CFG", "f32r_cbf16")


def _cfg(name):
    if name == "bf16":
        return bf16, bf16
    if name == "all_f32r":
        return f32r, f32r
    return f32r, bf16  # default: A/B f32r, C bf16


def build_nc(bpc: int = BPC, cfg_name: str | None = None):
    abdt, cdt = _cfg(cfg_name or MM_CFG)
    nc = bacc.Bacc(
        "TRN2",
        target_bir_lowering=False,
        debug=False,
        enable_asserts=False,
    )

    x_d = nc.dram_tensor("x", [bpc, C, S], abdt, kind="ExternalInput")
    xf_d = nc.dram_tensor("xf", [bpc, C, S], f32, kind="ExternalInput")
    # weight stripes pre-arranged on host: [chunk, c, 128]
    mt_d = nc.dram_tensor("mt", [KC, C, P], abdt, kind="ExternalInput")
    wpt_d = nc.dram_tensor("wpt", [KC, C, P], cdt, kind="ExternalInput")
    wvs_d = nc.dram_tensor("wvs", [C], f32, kind="ExternalInput")
    ones_d = nc.dram_tensor("ones", [P, P], abdt, kind="ExternalInput")
    bp_d = nc.dram_tensor("bp", [C], f32, kind="ExternalInput")
    out_d = nc.dram_tensor("out", [bpc, C, S], f32, kind="ExternalOutput")

    with tile.TileContext(nc) as tc:
        with (
            tc.tile_pool(name="weights", bufs=1) as wpool,
            tc.tile_pool(name="xc", bufs=10) as xpool,
            tc.tile_pool(name="xf", bufs=4) as xfpool,
            tc.tile_pool(name="y", bufs=1) as ypool,
            tc.tile_pool(name="h", bufs=10) as hpool,
            tc.tile_pool(name="vsb", bufs=2) as vpool,
            tc.tile_pool(name="vacc", bufs=2) as vaccpool,
            tc.tile_pool(name="vtmp", bufs=3) as vtmppool,
            tc.tile_pool(name="osb", bufs=4) as opool,
            tc.tile_pool(name="small", bufs=16) as spool,
            tc.tile_pool(name="psA", bufs=2, space="PSUM") as psA,
            tc.tile_pool(name="psB", bufs=2, space="PSUM") as psB,
            tc.tile_pool(name="psC", bufs=2, space="PSUM") as psC,
        ):
            # ---- small resident weights first (cheap DMAs) ----
            wvs_sb = wpool.tile([P, KC], f32, tag="wvs")
            nc.sync.dma_start(wvs_sb[:], wvs_d.rearrange("(ko ki) -> ki ko", ki=P))
            bp_sb = wpool.tile([P, KC], f32, tag="bp")
            nc.sync.dma_start(bp_sb[:], bp_d.rearrange("(o p) -> p o", p=P))
            ones_sb = wpool.tile([P, P], abdt, tag="ones")
            nc.sync.dma_start(ones_sb[:], ones_d[:, :])
            wpt_sb = wpool.tile([P, TC, C], cdt, tag="wpt")
            mt_sb = wpool.tile([P, KC, C], abdt, tag="mt")

            for b in range(bpc):
                # ---- load x chunks; order matches first-use order ----
                xc = []
                for k in range(KC):
                    t = xpool.tile([P, S], abdt, tag="xc")
                    xc.append(t)
                if b == 0:
                    # stripe 0 of mt, then the x half the first psum group
                    # needs, then the rest — so the PE starts ~7us in.
                    nc.sync.dma_start(
                        mt_sb[:, :, 0:P],
                        mt_d[0].rearrange("(ko ki) m -> ki ko m", ki=P),
                    )
                    for k in range(KC):
                        nc.sync.dma_start(xc[k][:, 0:NN], x_d[b, k * P : (k + 1) * P, 0:NN])
                    for k in range(KC):
                        nc.sync.dma_start(xc[k][:, NN:S], x_d[b, k * P : (k + 1) * P, NN:S])
                    for mc in range(1, KC):
                        nc.sync.dma_start(
                            mt_sb[:, :, mc * P : (mc + 1) * P],
                            mt_d[mc].rearrange("(ko ki) m -> ki ko m", ki=P),
                        )
                else:
                    for k in range(KC):
                        nc.sync.dma_start(t := xc[k][:], x_d[b, k * P : (k + 1) * P, :])

                # ---- stage A2a: vacc[p,s] = sum_k wvs[k*128+p] * x[k][p,s]
                # (products on ACT, accumulate on DVE; PE only does the
                #  final 128-partition contraction via a ones-matmul)
                vacc = vaccpool.tile([P, S], abdt, tag="vacc")
                nc.scalar.activation(
                    vacc[:], xc[0][:], mybir.ActivationFunctionType.Copy,
                    scale=wvs_sb[:, 0:1],
                )
                for k in range(1, KC):
                    vt = vtmppool.tile([P, S], f32, tag="vtmp")
                    nc.scalar.activation(
                        vt[:], xc[k][:], mybir.ActivationFunctionType.Copy,
                        scale=wvs_sb[:, k : k + 1],
                    )
                    nc.vector.tensor_tensor(
                        vacc[:], vacc[:], vt[:], mybir.AluOpType.add
                    )

                # ---- stage A: y = M x  (y[c',s]) ----
                y_sb = ypool.tile([P, KC, S], abdt, tag="y")
                for mc in range(KC):
                    for n in range(NCH):
                        ps = psA.tile([P, NN], f32, tag="psA")
                        for k in range(KC):
                            nc.tensor.matmul(
                                ps[:],
                                mt_sb[:, k, mc * P : (mc + 1) * P],
                                xc[k][:, n * NN : (n + 1) * NN],
                                start=(k == 0),
                                stop=(k == KC - 1),
                            )
                        nc.any.tensor_copy(
                            out=y_sb[:, mc, n * NN : (n + 1) * NN], in_=ps[:]
                        )

                # ---- stage A2b: vs broadcast via ones-matmul ----
                vsb = vpool.tile([P, S], cdt, tag="vsb")
                for n in range(NCH):
                    psv = psA.tile([P, NN], f32, tag="psA")
                    nc.tensor.matmul(
                        psv[:], ones_sb[:], vacc[:, n * NN : (n + 1) * NN],
                        start=True, stop=True,
                    )
                    nc.any.tensor_copy(out=vsb[:, n * NN : (n + 1) * NN], in_=psv[:])

                if b == 0:
                    # proj weights not needed until stage C; loading them here
                    # keeps the critical-path DMAs (mt, x) uncontended.
                    for oc in range(KC):
                        nc.sync.dma_start(
                            wpt_sb[:, :, oc * P : (oc + 1) * P],
                            wpt_d[oc].rearrange("(ko ki) m -> ki ko m", ki=P),
                        )

                # ---- stage B: logits tiles, fused softmax * vs ----
                hts = []
                for tt in range(TC):
                    psl = psB.tile([P, S], f32, tag="psB")
                    for n in range(NCH):
                        for k in range(KC):
                            nc.tensor.matmul(
                                psl[:, n * NN : (n + 1) * NN],
                                xc[k][:, tt * P : (tt + 1) * P],
                                y_sb[:, k, n * NN : (n + 1) * NN],
                                start=(k == 0),
                                stop=(k == KC - 1),
                            )
                    e = hpool.tile([P, S], cdt, tag="h")
                    rs = spool.tile([P, 1], f32, tag="rs")
                    # e = exp(logits / sqrt(C)); rs = row sums (free on ACT)
                    nc.scalar.activation(
                        e[:], psl[:], mybir.ActivationFunctionType.Exp,
                        scale=float(SCALE), accum_out=rs[:],
                    )
                    rcp = spool.tile([P, 1], f32, tag="rcp")
                    nc.vector.reciprocal(rcp[:], rs[:])
                    # normalize rows (per-partition scalar) on ACT
                    nc.scalar.activation(
                        e[:], e[:], mybir.ActivationFunctionType.Copy,
                        scale=rcp[:],
                    )
                    # h = attn * vs  (vs broadcast over partitions via vsb)
                    nc.vector.tensor_tensor(
                        e[:], e[:], vsb[:], mybir.AluOpType.mult
                    )
                    hts.append(e)

                # ---- stage C: out = w_proj @ h + x + b ----
                for oc in range(KC):
                    for n in range(NCH):
                        pso = psC.tile([P, NN], f32, tag="psC")
                        for tt in range(TC):
                            nc.tensor.matmul(
                                pso[:],
                                wpt_sb[:, tt, oc * P : (oc + 1) * P],
                                hts[tt][:, n * NN : (n + 1) * NN],
                                start=(tt == 0),
                                stop=(tt == TC - 1),
                            )
                        xf_t = xfpool.tile([P, NN], f32, tag="xf")
                        nc.sync.dma_start(
                            xf_t[:],
                            xf_d[b, oc * P : (oc + 1) * P, n * NN : (n + 1) * NN],
                        )
                        osb = opool.tile([P, NN], f32, tag="osb")
                        nc.vector.tensor_tensor(
                            osb[:], pso[:], xf_t[:], mybir.AluOpType.add
                        )
                        nc.vector.tensor_scalar(
                            osb[:], osb[:], bp_sb[:, oc : oc + 1], None,
                            mybir.AluOpType.add,
                        )
                        nc.sync.dma_start(
                            out_d[b, oc * P : (oc + 1) * P, n * NN : (n + 1) * NN],
                            osb[:],
                        )
    nc.compile()
    return nc


def _round_f32r(a):
    """Round fp32 to float32r (11-bit mantissa, round-to-nearest-even)."""
    u = np.ascontiguousarray(a.astype(np.float32)).view(np.uint32)
    lsb = (u >> np.uint32(12)) & np.uint32(1)
    r = (u + np.uint32(0x7FF) + lsb) & np.uint32(0xFFFFF000)
    return r.view(np.float32)


def _to_dt(a, dt):
    if dt == bf16:
        return np.ascontiguousarray(a).astype(ml_dtypes.bfloat16)
    if dt == f32r:
        return _round_f32r(a)
    return np.ascontiguousarray(a).astype(np.float32)


def _host_prep(w_qkv, w_proj, b_proj, cfg_name):
    abdt, cdt = _cfg(cfg_name)
    wq = w_qkv[0:C].astype(np.float64)
    wk = w_qkv[C : 2 * C].astype(np.float64)
    wv = w_qkv[2 * C : 3 * C]
    # lhsT for y-matmul: MT[c, c'] = M[c', c],  M = Wq^T Wk  =>  MT = Wk^T Wq
    mt = np.ascontiguousarray(wk.T @ wq).astype(np.float32)
    wvs = wv.sum(axis=0, dtype=np.float64).astype(np.float32)
    wpt = np.ascontiguousarray(w_proj.T).astype(np.float32)
    # stripe layout [chunk, c, 128]
    mt_s = np.ascontiguousarray(mt.reshape(C, KC, P).transpose(1, 0, 2))
    wpt_s = np.ascontiguousarray(wpt.reshape(C, KC, P).transpose(1, 0, 2))
    return _to_dt(mt_s, abdt), _to_dt(wpt_s, cdt), wvs, b_proj.astype(np.float32)


_NC_CACHE = {}


def _get_nc(bpc=BPC, cfg_name=None):
    key = (bpc, cfg_name or MM_CFG)
    if key not in _NC_CACHE:
        _NC_CACHE[key] = build_nc(bpc, cfg_name)
    return _NC_CACHE[key]


def kernel(x, w_qkv, w_proj, b_proj, _trace=False):
    cfg_name = MM_CFG
    abdt, _ = _cfg(cfg_name)
    x = np.asarray(x, dtype=np.float32)
    mt, wpt, wvs, bp = _host_prep(
        np.asarray(w_qkv, np.float32),
        np.asarray(w_proj, np.float32),
        np.asarray(b_proj, np.float32),
        cfg_name,
    )
    xr_full = x.reshape(B, C, S)
    x_mm = _to_dt(xr_full, abdt)
    in_maps = []
    for c in range(N_CORES):
        sl = slice(c * BPC, (c + 1) * BPC)
        in_maps.append(
            {
                "x": np.ascontiguousarray(x_mm[sl]),
                "xf": np.ascontiguousarray(xr_full[sl]),
                "mt": mt,
                "wpt": wpt,
                "wvs": wvs,
                "ones": _to_dt(np.ones((P, P), np.float32), abdt),
                "bp": bp,
            }
        )
    nc = _get_nc(BPC, cfg_name)
    res = run_bass_kernel_spmd(
        nc, in_maps, core_ids=list(range(N_CORES)), trace=_trace
    )
    out = np.concatenate([r["out"] for r in res.results], axis=0)
    out = out.reshape(B, C, HH, WW)
    if _trace:
        kernel.last_results = res
    return out
